# revision 33
# baseline (speedup 1.0000x reference)
"""Multi-head attention (B=2, S=4096, E=512, H=8) on 8 Trainium2 cores.

Sharding: one (batch, head-pair) unit per core — core c handles batch c//4
and heads 2*(c%4), 2*(c%4)+1.  Each core runs the full pipeline for its two
heads: QKV projection, flash-style attention (no S^2 materialization in
DRAM), and its partial output projection (Wo row-slice).  The host sums the
four partials per batch and adds the fused bias (bo + bv @ Wo).

Engine budget per core (the kernel is ScalarE-bound):
  - exp of all 2*S^2 logits runs on the Activation engine (~0.93 ns/col);
    everything else is kept off it: q/k biases are added on the DVE, the
    denominator reciprocal uses the fast DVE approx, PSUM->SBUF staging is
    DVE/DMA.
  - x/W inputs ship as bf16 (halves the HBM floor for the K/V prefetch
    phase); all on-chip intermediates stay f32(r).
  - logits matmuls contract K=64 per head at row groups 0/64 so head pairs
    run concurrently on the PE array; the flash loop is emitted in
    two-group batches so pairs stay adjacent in the PE queue.
  - softmax denominators ride as a 65th column of each head's V tile
    (attn@V and the denominator come out of the same matmul), with the
    additive mask folded in as a multiplicative per-key weight
    w_k = exp(-1e9 * mask_k) applied to that V tile.
"""

import numpy as np
import ml_dtypes
from contextlib import ExitStack

import concourse.bass as bass
import concourse.bacc as bacc
import concourse.tile as tile
from concourse import mybir
from concourse.bass_utils import run_bass_kernel_spmd

F32 = mybir.dt.float32
F32R = mybir.dt.float32r
BF16 = mybir.dt.bfloat16
BF16NP = ml_dtypes.bfloat16

B = 2
S = 4096
E = 512
H = 8
D = 64
NCORES = 8
HPC = 2            # heads per core
DH = HPC * D       # 128
SQ = 512           # q-block (matmul moving free dim)
SKB = 128          # k-block (one partition tile)
ET = E // 128      # e-tiles in the contraction
EXP_SLOTS = 2      # sk-slots per exp batch ([128, 1024] activations)
LOADW = 1024       # input DMA block width (bf16 -> 2KB lines)
WARM = True        # split first K chunk for an early exp start
BOUNDARY_BATCH = True  # overlap last/first flash groups across q-blocks

_NC_CACHE = {}


def _build_kernel(ctx, tc, s, reps=1):
    nc = tc.nc

    xqT = nc.declare_dram_parameter("xqT", [E, s], BF16, isOutput=False)
    xkT = nc.declare_dram_parameter("xkT", [E, s], BF16, isOutput=False)
    xvT = nc.declare_dram_parameter("xvT", [E, s], BF16, isOutput=False)
    wq = nc.declare_dram_parameter("wq", [E, DH], BF16, isOutput=False)
    wk = nc.declare_dram_parameter("wk", [E, DH], BF16, isOutput=False)
    wv = nc.declare_dram_parameter("wv", [E, DH], BF16, isOutput=False)
    wo = nc.declare_dram_parameter("wo", [DH, E], F32, isOutput=False)
    bq = nc.declare_dram_parameter("bq", [DH], F32, isOutput=False)
    bk = nc.declare_dram_parameter("bk", [DH], F32, isOutput=False)
    wm = nc.declare_dram_parameter("wm", [s], F32, isOutput=False)
    out = nc.declare_dram_parameter("out", [E, s], F32, isOutput=True)

    const = ctx.enter_context(tc.tile_pool(name="const", bufs=1))
    res = ctx.enter_context(tc.tile_pool(name="res", bufs=1))

    nsk = s // SKB

    # Weights / biases / mask weights resident in SBUF
    wq_sb = const.tile([128, ET, DH], BF16)
    nc.sync.dma_start(wq_sb[:], wq.rearrange("(t p) d -> p t d", p=128))
    wk_sb = const.tile([128, ET, DH], BF16)
    nc.sync.dma_start(wk_sb[:], wk.rearrange("(t p) d -> p t d", p=128))
    bq_sb = const.tile([128, 1], F32)
    nc.sync.dma_start(bq_sb[:], bq.rearrange("(p o) -> p o", o=1))
    bk_sb = const.tile([128, 1], F32)
    nc.sync.dma_start(bk_sb[:], bk.rearrange("(p o) -> p o", o=1))
    wv_sb = const.tile([128, ET, DH], BF16)
    wo_sb = const.tile([128, E], F32R)
    wm_sb = const.tile([128, nsk], F32)
    # ones row (at partition D) used as lhsT of the K=1 broadcast matmul
    # (memset can't target f32r; copy-rounding from an f32 staging tile can)
    ones_f32 = const.tile([128, D], F32)
    nc.vector.memset(ones_f32[:], 1.0)
    ones_bc = const.tile([128, D], F32R)
    nc.vector.tensor_copy(ones_bc[:], ones_f32[:])

    # Resident K^T (d-major) and V (s-major, with w/ones column per head)
    khT = res.tile([128, s], F32R)
    vh = res.tile([128, nsk, 2 * (D + 1)], F32R)

    def emit_late_consts():
        # deferred so the first xq/xk input chunks win the DMA queue
        nc.sync.dma_start(wv_sb[:], wv.rearrange("(t p) d -> p t d", p=128))
        nc.sync.dma_start(wm_sb[:], wm.rearrange("(t p) -> p t", p=128))
        nc.sync.dma_start(wo_sb[:], wo[:, :].bitcast(F32R))
        # w/ones columns of vh (col 64 = head0, col 129 = head1)
        nc.vector.tensor_copy(vh[:, :, D], wm_sb[:, :])
        nc.vector.tensor_copy(vh[:, :, 2 * D + 1], wm_sb[:, :])

    xkv_pool = ctx.enter_context(tc.tile_pool(name="xkv", bufs=3))

    env = dict(locals())
    for _rep in range(reps):
        _phase_ab(tc, s, env)


def _phase_ab(tc, s, env):
    nc = tc.nc
    AF = mybir.ActivationFunctionType
    (xqT, xkT, xvT, wq_sb, wk_sb, wv_sb, wo_sb, bq_sb, bk_sb, wm_sb,
     ones_bc, khT, vh, xkv_pool, out, emit_late_consts) = (
        env["xqT"], env["xkT"], env["xvT"], env["wq_sb"], env["wk_sb"],
        env["wv_sb"], env["wo_sb"], env["bq_sb"], env["bk_sb"], env["wm_sb"],
        env["ones_bc"], env["khT"], env["vh"], env["xkv_pool"], env["out"],
        env["emit_late_consts"])

    nsq = s // SQ
    nsk = s // SKB
    loadw = min(LOADW, s)
    nload = s // loadw
    kb_per_chunk = loadw // SKB

    bctx = ExitStack()
    lg_pool = bctx.enter_context(tc.tile_pool(name="lg", bufs=2, space="PSUM"))
    acc_pool = bctx.enter_context(tc.tile_pool(name="acc", bufs=4, space="PSUM"))
    exp_pool = bctx.enter_context(tc.tile_pool(name="expp", bufs=5))
    qh_pool = bctx.enter_context(tc.tile_pool(name="qh", bufs=2))
    o_pool = bctx.enter_context(tc.tile_pool(name="o", bufs=2))
    sm_pool = bctx.enter_context(tc.tile_pool(name="sm", bufs=4))

    xkT_r = xkT.rearrange("(t p) s -> p t s", p=128)
    xvT_r = xvT.rearrange("(t p) s -> p t s", p=128)
    xqT_r = xqT.rearrange("(t p) s -> p t s", p=128)
    out_r = out.rearrange("(t p) s -> p t s", p=128)
    qper = loadw // SQ   # q-blocks per xq load

    def emit_kv_chunk0_warm():
        # Chunk 0, reordered for fastest first-exp: half of K first (so the
        # first logits groups can start), then xq (queued behind it), then
        # the rest of K and all of V.
        lsl = slice(0, loadw)
        xk_t = xkv_pool.tile([128, ET, loadw], BF16, tag="xkv")
        halves = loadw // SQ
        def kproj_half(half):
            hsl = slice(half * SQ, (half + 1) * SQ)
            pk_t = lg_pool.tile([128, EXP_SLOTS, SQ], F32, tag="lg")
            pk = pk_t[:, 0, :]
            for et in range(ET):
                nc.tensor.matmul(
                    pk,
                    lhsT=wk_sb[:, et, :],
                    rhs=xk_t[:, et, hsl],
                    start=(et == 0),
                    stop=(et == ET - 1),
                )
            nc.vector.tensor_scalar_add(khT[:, hsl], pk, bk_sb[:, 0:1])
        nc.sync.dma_start(xk_t[:, :, 0:SQ], xkT_r[:, :, 0:SQ])
        kproj_half(0)
        qh0 = emit_head(0)
        for half in range(1, halves):
            hsl = slice(half * SQ, (half + 1) * SQ)
            nc.sync.dma_start(xk_t[:, :, hsl], xkT_r[:, :, hsl])
            kproj_half(half)
        xv_t = xkv_pool.tile([128, ET, loadw], BF16, tag="xkv")
        nc.sync.dma_start(xv_t[:], xvT_r[:, :, lsl])
        emit_late_consts()
        pv_t = lg_pool.tile([128, EXP_SLOTS, SQ], F32, tag="lg")
        per_slot = SQ // DH
        for sub in range(loadw // SKB):
            pv = pv_t[:, sub // per_slot, (sub % per_slot) * DH:(sub % per_slot + 1) * DH]
            for et in range(ET):
                nc.tensor.matmul(
                    pv,
                    lhsT=xv_t[:, et, sub * SKB:(sub + 1) * SKB],
                    rhs=wv_sb[:, et, :],
                    start=(et == 0),
                    stop=(et == ET - 1),
                )
            wcol = wm_sb[:, sub:sub + 1]
            nc.vector.tensor_scalar_mul(vh[:, sub, 0:D], pv[:, 0:D], wcol)
            nc.vector.tensor_scalar_mul(vh[:, sub, D + 1:2 * D + 1], pv[:, D:DH], wcol)
        return qh0

    def emit_kv_chunk(blk):
        """Load one 1024-wide x chunk and project it into khT / vh.  PSUM
        comes from the lg ring (pk: one tile's two slots; pv: packed 128-col
        slices of another tile) so phase A and the flash loop share banks
        and overlap freely."""
        lsl = slice(blk * loadw, (blk + 1) * loadw)
        xk_t = xkv_pool.tile([128, ET, loadw], BF16, tag="xkv")
        nc.sync.dma_start(xk_t[:], xkT_r[:, :, lsl])
        xv_t = xkv_pool.tile([128, ET, loadw], BF16, tag="xkv")
        nc.sync.dma_start(xv_t[:], xvT_r[:, :, lsl])
        pk_t = lg_pool.tile([128, EXP_SLOTS, SQ], F32, tag="lg")
        for half in range(loadw // SQ):
            hsl = slice(half * SQ, (half + 1) * SQ)
            osl = slice(blk * loadw + half * SQ, blk * loadw + (half + 1) * SQ)
            pk = pk_t[:, half % EXP_SLOTS, :]
            for et in range(ET):
                nc.tensor.matmul(
                    pk,
                    lhsT=wk_sb[:, et, :],
                    rhs=xk_t[:, et, hsl],
                    start=(et == 0),
                    stop=(et == ET - 1),
                )
            nc.vector.tensor_scalar_add(khT[:, osl], pk, bk_sb[:, 0:1])
        pv_t = lg_pool.tile([128, EXP_SLOTS, SQ], F32, tag="lg")
        per_slot = SQ // DH
        for sub in range(loadw // SKB):
            s32 = blk * kb_per_chunk + sub
            pv = pv_t[:, sub // per_slot, (sub % per_slot) * DH:(sub % per_slot + 1) * DH]
            for et in range(ET):
                nc.tensor.matmul(
                    pv,
                    lhsT=xv_t[:, et, sub * SKB:(sub + 1) * SKB],
                    rhs=wv_sb[:, et, :],
                    start=(et == 0),
                    stop=(et == ET - 1),
                )
            wcol = wm_sb[:, s32:s32 + 1]
            nc.vector.tensor_scalar_mul(vh[:, s32, 0:D], pv[:, 0:D], wcol)
            nc.vector.tensor_scalar_mul(vh[:, s32, D + 1:2 * D + 1], pv[:, D:DH], wcol)

    pending_tail = None

    def make_tail(acc0, acc1, ostage, tmp1, sqsl):
        def emit_tail():
            bct = lg_pool.tile([128, EXP_SLOTS, SQ], F32, tag="lg")
            for h, acc in ((0, acc0), (1, acc1)):
                # rcp lives at partition D so the DVE reciprocal stays
                # partition-aligned with the denominator row of acc
                rcp = sm_pool.tile([128, SQ], F32R, tag="rcp")
                with nc.allow_low_precision(reason="f32r bits == f32 bits; rounding is a no-op here"):
                    nc.vector.reciprocal(rcp[D:D + 1, :], acc[D:D + 1, :])
                # broadcast 1/denom across D partitions with a K=1 matmul
                nc.tensor.matmul(
                    bct[0:D, h, :],
                    lhsT=ones_bc[D:D + 1, :],
                    rhs=rcp[D:D + 1, :],
                    start=True,
                    stop=True,
                )
                bc = sm_pool.tile([64, SQ], F32, tag="bc")
                nc.vector.tensor_copy(bc[:], bct[0:D, h, :])
                if h == 0:
                    nc.vector.tensor_mul(ostage[0:D, :], acc[0:D, :], bc[:])
                else:
                    # DVE lanes can't shift partitions; head1 rides SBUF DMA
                    nc.vector.tensor_mul(tmp1[:], acc[0:D, :], bc[:])
                    nc.sync.dma_start(ostage[D:DH, :], tmp1[:])
            # Output projection (rows of Wo for this core's heads) as M=64
            # column-group pairs into the now-dead acc banks: no PSUM
            # allocations in the tail at all.
            for m in range(ET):
                pp = (acc0 if m % 2 == 0 else acc1)[:, :]
                nc.tensor.matmul(
                    pp,
                    lhsT=wo_sb[:, m * 128:(m + 1) * 128],
                    rhs=ostage[:],
                    start=True,
                    stop=True,
                )
                ot = o_pool.tile([128, SQ], F32, tag="ot")
                nc.vector.tensor_copy(ot[:], pp)
                nc.sync.dma_start(out_r[:, m, sqsl], ot[:])
        return emit_tail

    chunks_emitted = 0
    xq_state = [None]

    def emit_head(sqi):
        # xq load + Q projection + bias for block sqi (slot 0 of an lg tile)
        if sqi % qper == 0:
            lsl = slice(sqi * SQ, sqi * SQ + loadw)
            xq_new = xkv_pool.tile([128, ET, loadw], BF16, tag="xq")
            nc.sync.dma_start(xq_new[:], xqT_r[:, :, lsl])
            xq_state[0] = xq_new
        qsl = slice((sqi % qper) * SQ, (sqi % qper + 1) * SQ)
        lgq_t = lg_pool.tile([128, EXP_SLOTS, SQ], F32, tag="lg")
        lgq = lgq_t[:, 0, :]
        for et in range(ET):
            nc.tensor.matmul(
                lgq,
                lhsT=wq_sb[:, et, :],
                rhs=xq_state[0][:, et, qsl],
                start=(et == 0),
                stop=(et == ET - 1),
            )
        qh_t = qh_pool.tile([128, SQ], F32R)
        nc.vector.tensor_scalar_add(qh_t[:], lgq, bq_sb[:, 0:1])
        return qh_t

    next_qh = None
    pre_state = None
    for sqi in range(nsq):
        sqsl = slice(sqi * SQ, (sqi + 1) * SQ)
        if sqi == 0:
            if WARM:
                qh_t = emit_kv_chunk0_warm()
                chunks_emitted = 1
            else:
                qh_t = emit_head(0)
        else:
            qh_t = next_qh
        next_qh = None

        # Flash loop: one group = one k-block for both heads, so the two
        # K=64 logits matmuls are adjacent in the PE queue (row groups 0/64
        # -> concurrent on HW).  During block 0 the K/V projection chunks
        # are emitted just-in-time between groups so the exp stream starts
        # as soon as the first chunk lands.  The previous block's tail is
        # deferred into this block's flash so its PE ops never
        # head-of-line-block while waiting on the DVE reciprocal chain, and
        # the last two groups are batched with the next block's first two
        # (logits+exp ahead of the AVs) so the exp stream never drains at
        # block boundaries.
        if pre_state is None:
            acc0 = acc_pool.tile([128, SQ], F32, tag="acc")
            acc1 = acc_pool.tile([128, SQ], F32, tag="acc")
            k_start = 0
        else:
            acc0, acc1 = pre_state
            pre_state = None
            k_start = 2

        def g_logits_exp(qh, k):
            lg = lg_pool.tile([128, EXP_SLOTS, SQ], F32, tag="lg")
            for h in range(HPC):
                nc.tensor.matmul(
                    lg[:, h, :],
                    lhsT=khT[h * D:(h + 1) * D, k * SKB:(k + 1) * SKB],
                    rhs=qh[h * D:(h + 1) * D, :],
                    start=True,
                    stop=True,
                )
            ex = exp_pool.tile([128, EXP_SLOTS, SQ], F32R, tag="ex")
            nc.scalar.activation(ex[:], lg[:], AF.Exp, scale=0.125)
            return ex

        def g_av(ex, a0, a1, k):
            for h in range(HPC):
                acc = a0 if h == 0 else a1
                nc.tensor.matmul(
                    acc[0:D + 1, :],
                    lhsT=vh[:, k, h * (D + 1):(h + 1) * (D + 1)],
                    rhs=ex[:, h, :],
                    start=(k == 0),
                    stop=(k == nsk - 1),
                )

        for k in range(k_start, nsk):
            while chunks_emitted < nload and k + 4 >= chunks_emitted * kb_per_chunk:
                emit_kv_chunk(chunks_emitted)
                chunks_emitted += 1
            if BOUNDARY_BATCH and nsk >= 16 and k == nsk - 2 and sqi + 1 < nsq:
                exA = g_logits_exp(qh_t, k)
                exB = g_logits_exp(qh_t, k + 1)
                exC = g_logits_exp(next_qh, 0)
                exD = g_logits_exp(next_qh, 1)
                g_av(exA, acc0, acc1, k)
                g_av(exB, acc0, acc1, k + 1)
                nacc0 = acc_pool.tile([128, SQ], F32, tag="acc")
                nacc1 = acc_pool.tile([128, SQ], F32, tag="acc")
                g_av(exC, nacc0, nacc1, 0)
                g_av(exD, nacc0, nacc1, 1)
                pre_state = (nacc0, nacc1)
                break
            ex = g_logits_exp(qh_t, k)
            g_av(ex, acc0, acc1, k)
            if k == 2 and pending_tail is not None:
                pending_tail()
                pending_tail = None
            if k == max(2, nsk - 8) and sqi + 1 < nsq:
                next_qh = emit_head(sqi + 1)

        if pending_tail is not None:   # nsq==1 or very short flash
            pending_tail()
        ostage = o_pool.tile([128, SQ], F32R, tag="onorm")
        tmp1 = o_pool.tile([64, SQ], F32R, tag="tmp1")
        pending_tail = make_tail(acc0, acc1, ostage, tmp1, sqsl)

    pending_tail()

    bctx.close()


def build_nc(s=S, reps=1):
    key = (s, reps)
    if key in _NC_CACHE:
        return _NC_CACHE[key]
    nc = bacc.Bacc("TRN2", target_bir_lowering=False, debug=False)
    with tile.TileContext(nc) as tc:
        with ExitStack() as ctx:
            _build_kernel(ctx, tc, s, reps=reps)
    nc.compile()
    _NC_CACHE[key] = nc
    return nc


def make_in_maps(q, k, v, mask, Wq, bq, Wk, bk, Wv, bv, Wo, bo):
    q = np.asarray(q, np.float32)
    k = np.asarray(k, np.float32)
    v = np.asarray(v, np.float32)
    mask = np.asarray(mask, np.float32)
    Wq = np.asarray(Wq, np.float32)
    Wk = np.asarray(Wk, np.float32)
    Wv = np.asarray(Wv, np.float32)
    Wo = np.asarray(Wo, np.float32)
    bq = np.asarray(bq, np.float32)
    bk = np.asarray(bk, np.float32)

    xT = {}
    wmb = {}
    for b in range(q.shape[0]):
        xT[("q", b)] = np.ascontiguousarray(q[b].T).astype(BF16NP)
        xT[("k", b)] = np.ascontiguousarray(k[b].T).astype(BF16NP)
        xT[("v", b)] = np.ascontiguousarray(v[b].T).astype(BF16NP)
        # additive mask -> exact multiplicative per-key weight
        wmb[b] = np.exp(np.float32(-1e9) * mask[b, 0, 0, :]).astype(np.float32)

    in_maps = []
    for c in range(NCORES):
        b = c // (NCORES // B)
        p = c % (NCORES // B)
        hsl = slice(p * DH, (p + 1) * DH)
        in_maps.append({
            "xqT": xT[("q", b)],
            "xkT": xT[("k", b)],
            "xvT": xT[("v", b)],
            "wq": np.ascontiguousarray(Wq[:, hsl]).astype(BF16NP),
            "wk": np.ascontiguousarray(Wk[:, hsl]).astype(BF16NP),
            "wv": np.ascontiguousarray(Wv[:, hsl]).astype(BF16NP),
            "wo": np.ascontiguousarray(Wo[hsl, :]),
            "bq": np.ascontiguousarray(bq[hsl]),
            "bk": np.ascontiguousarray(bk[hsl]),
            "wm": wmb[b],
        })
    return in_maps


def gather(results, bv, bo, Wo):
    bias_total = (np.asarray(bv, np.float32) @ np.asarray(Wo, np.float32)
                  + np.asarray(bo, np.float32))
    cpb = NCORES // B
    full = np.empty((B, S, E), np.float32)
    for b in range(B):
        acc = results[b * cpb]["out"].astype(np.float32, copy=True)
        for c in range(b * cpb + 1, (b + 1) * cpb):
            acc += results[c]["out"]
        full[b] = acc.T + bias_total
    return full


def run(trace=False, **inputs):
    nc = build_nc(S)
    in_maps = make_in_maps(
        inputs["q"], inputs["k"], inputs["v"], inputs["mask"],
        inputs["Wq"], inputs["bq"], inputs["Wk"], inputs["bk"],
        inputs["Wv"], inputs["bv"], inputs["Wo"], inputs["bo"],
    )
    res = run_bass_kernel_spmd(nc, in_maps, list(range(NCORES)), trace=trace)
    out = gather(res.results, inputs["bv"], inputs["bo"], inputs["Wo"])
    return out, res


def kernel(**inputs):
    out, _ = run(trace=False, **inputs)
    return out
